# revision 1
# baseline (speedup 1.0000x reference)
"""SnakeHead Trainium2 kernel.

Model (per batch): bilinear-sample a [256,256,126] feature map at 1024
vertices, concat the (y,x) coords -> [1024,128], 1x1 conv to 512 + ReLU,
six dilated (1,3,9,9,3,1) kernel-3 conv1d layers 512->512 + ReLU, final
1x1 conv 512->2.

Strategy: data-parallel over batch, 2 batches per NeuronCore (16/8).
On each core:
  - gather the 4 bilinear corners with indirect DMA (row pairs: corners
    (y0,x0)/(y0,x0+1) are contiguous in DRAM -> 1008B descriptors)
  - bilinear-combine on the vector engine, vertices-on-partitions
  - PE-transpose to channel-major [ch, tok] layout
  - all convs as float32r matmuls (full PE rate), channels on
    partitions, tokens on the free axis; dilated taps are just shifted
    rhs slices into zero-padded halo activation buffers
  - bias+ReLU fused in the scalar-engine PSUM->SBUF eviction
"""

import numpy as np
from contextlib import ExitStack

import concourse.bass as bass
import concourse.bacc as bacc
import concourse.mybir as mybir
import concourse.tile as tile
from concourse.bass import IndirectOffsetOnAxis
from concourse.bass_utils import run_bass_kernel_spmd
from concourse.masks import make_identity

P = 128
B, N, H, W, Cf, Ch = 16, 1024, 256, 256, 126, 512
NCORES = 8
BPC = B // NCORES          # batches per core
T = BPC * N                # tokens per core
D = Cf + 2                 # input channels to layer 0
DILS = (1, 3, 9, 9, 3, 1)
PAD = 16                   # halo >= max dilation (9)
SEG = PAD + N + PAD        # per-batch activation columns
NT = T // P                # 128-token tiles per core
CB = Ch // P               # 128-channel blocks (4)
HALF = 512                 # matmul moving-dim tile (tokens)

F32 = mybir.dt.float32
FR = mybir.dt.float32r
I32 = mybir.dt.int32
AF = mybir.ActivationFunctionType
ALU = mybir.AluOpType


def build_program(reps=1, nlayers=6, wbufs=2):
    nc = bacc.Bacc(trn_type="TRN2", target_bir_lowering=False)

    verts = nc.declare_dram_parameter("verts", [P, BPC * (N // P) * 2], F32, False)
    fm = nc.declare_dram_parameter("fm", [BPC * H * W, Cf], F32, False)
    w0 = nc.declare_dram_parameter("w0", [P, Ch], FR, False)
    b0 = nc.declare_dram_parameter("b0", [P, CB], F32, False)
    ws = nc.declare_dram_parameter("ws", [6, P, 3 * CB * Ch], FR, False)
    bs = nc.declare_dram_parameter("bs", [P, 6 * CB], F32, False)
    woff = nc.declare_dram_parameter("woff", [P, CB * 2], FR, False)
    out = nc.declare_dram_parameter("out", [2, T], F32, True)

    with tile.TileContext(nc) as tc, ExitStack() as ctx:
        const = ctx.enter_context(tc.tile_pool(name="const", bufs=1))
        work = ctx.enter_context(tc.tile_pool(name="work", bufs=1))
        gpool = ctx.enter_context(tc.tile_pool(name="gpool", bufs=4))
        wpool = ctx.enter_context(tc.tile_pool(name="wpool", bufs=wbufs))
        hpool = ctx.enter_context(tc.tile_pool(name="hpool", bufs=1))
        psum = ctx.enter_context(tc.tile_pool(name="psum", bufs=4, space="PSUM"))
        for _ in range(reps):
            _emit_body(nc, tc, const, work, gpool, wpool, hpool, psum,
                       verts, fm, w0, b0, ws, bs, woff, out, nlayers)

    # Clear all kernel semaphores + DMA queues at the tail so the loaded
    # NEFF can be re-executed: without this, a second nrt_execute starts
    # with end-of-run semaphore values and every wait is pre-satisfied.
    nc.reset()
    nc.finalize()
    return nc


def _emit_body(nc, tc, const, work, gpool, wpool, hpool, psum,
               verts, fm, w0, b0, ws, bs, woff, out, nlayers=6):
    if True:
        # ---- constants / small loads ----
        ident = const.tile([P, P], F32)
        make_identity(nc, ident[:])
        v_sb = const.tile([P, BPC * (N // P) * 2], F32)   # [p, (b c) yx]
        nc.sync.dma_start(out=v_sb[:], in_=verts[:])
        w0_sb = const.tile([P, Ch], FR)
        nc.sync.dma_start(out=w0_sb[:], in_=w0[:])
        b0_sb = const.tile([P, CB], F32)
        nc.sync.dma_start(out=b0_sb[:], in_=b0[:])
        bs_sb = const.tile([P, 6 * CB], F32)
        nc.sync.dma_start(out=bs_sb[:], in_=bs[:])
        woff_sb = const.tile([P, CB * 2], FR)
        nc.sync.dma_start(out=woff_sb[:], in_=woff[:])

        # ---- vertex math: coords, floor, weights, flat indices ----
        v3 = v_sb[:].rearrange("p (j t) -> p j t", t=2)       # [128, 16, 2]
        cyx = work.tile([P, NT, 2], F32)
        # coords = (v+1) * (dim-1)/2
        nc.vector.tensor_scalar(
            out=cyx[:], in0=v3, scalar1=127.5, scalar2=127.5,
            op0=ALU.mult, op1=ALU.add)
        yi = work.tile([P, NT, 2], I32)
        nc.vector.tensor_copy(yi[:], cyx[:])                   # unknown rounding
        yf = work.tile([P, NT, 2], F32)
        nc.vector.tensor_copy(yf[:], yi[:])
        gt = work.tile([P, NT, 2], F32)
        nc.vector.tensor_tensor(out=gt[:], in0=yf[:], in1=cyx[:], op=ALU.is_gt)
        y0 = work.tile([P, NT, 2], F32)                        # floor(coords)
        nc.vector.tensor_tensor(out=y0[:], in0=yf[:], in1=gt[:], op=ALU.subtract)
        wyx = work.tile([P, NT, 2], F32)                       # frac part
        nc.vector.tensor_tensor(out=wyx[:], in0=cyx[:], in1=y0[:], op=ALU.subtract)

        uyx = work.tile([P, NT, 2], F32)                       # 1 - frac
        nc.vector.tensor_scalar(
            out=uyx[:], in0=wyx[:], scalar1=-1.0, scalar2=1.0,
            op0=ALU.mult, op1=ALU.add)
        wy, wx = wyx[:, :, 0], wyx[:, :, 1]
        uy, ux = uyx[:, :, 0], uyx[:, :, 1]
        w00 = work.tile([P, NT], F32)
        w01 = work.tile([P, NT], F32)
        w10 = work.tile([P, NT], F32)
        w11 = work.tile([P, NT], F32)
        nc.vector.tensor_tensor(out=w00[:], in0=uy, in1=ux, op=ALU.mult)
        nc.vector.tensor_tensor(out=w01[:], in0=uy, in1=wx, op=ALU.mult)
        nc.vector.tensor_tensor(out=w10[:], in0=wy, in1=ux, op=ALU.mult)
        nc.vector.tensor_tensor(out=w11[:], in0=wy, in1=wx, op=ALU.mult)

        idxf = work.tile([P, NT], F32)                         # y0*W + x0 (+ batch)
        nc.vector.scalar_tensor_tensor(
            out=idxf[:], in0=y0[:, :, 0], scalar=float(W), in1=y0[:, :, 1],
            op0=ALU.mult, op1=ALU.add)
        for b in range(1, BPC):
            sl = idxf[:, b * (N // P):(b + 1) * (N // P)]
            nc.vector.tensor_scalar_add(out=sl, in0=sl, scalar1=float(b * H * W))
        # four corner indices: (y0,x0), (y0,x0+1), (y1,x0), (y1,x0+1)
        idx_c = []
        for ci_, delta in enumerate((0.0, 1.0, float(W), float(W + 1))):
            idxd = work.tile([P, NT], F32, name=f"idxd{ci_}")
            nc.vector.tensor_scalar_add(out=idxd[:], in0=idxf[:], scalar1=delta)
            idxi = work.tile([P, NT], I32, name=f"idxi{ci_}")
            nc.vector.tensor_copy(idxi[:], idxd[:])
            idx_c.append(idxi)

        # ---- activation halo buffers (ping/pong) ----
        h = [[[hpool.tile([P, SEG], FR, name=f"h{g}_{ci}_{b}", tag=f"h{g}_{ci}_{b}")
               for b in range(BPC)] for ci in range(CB)] for g in range(2)]
        zeros_f32 = const.tile([P, PAD], F32)
        nc.vector.memset(zeros_f32[:], 0.0)
        for g in range(2):
            for ci in range(CB):
                for b in range(BPC):
                    nc.vector.tensor_copy(h[g][ci][b][:, 0:PAD], zeros_f32[:])
                    nc.vector.tensor_copy(h[g][ci][b][:, PAD + N:SEG], zeros_f32[:])

        # ---- layer weight prefetch (double buffered) ----
        wcur = []
        for li in range(nlayers):
            wt = wpool.tile([P, 3 * CB * Ch], FR, tag="wlayer")
            nc.sync.dma_start(out=wt[:], in_=ws[li])
            wcur.append(wt)

        # ---- gather + bilinear + transpose into x_in [128ch, T] ----
        x_in = const.tile([P, T], FR)
        wcorner = (w00, w01, w10, w11)
        for j in range(NT):
            b = j // (N // P)
            cors = []
            for q in range(4):
                cq = gpool.tile([P, Cf], F32, name=f"cor{q}", tag=f"cor{q}")
                nc.gpsimd.indirect_dma_start(
                    out=cq[:], out_offset=None, in_=fm[:],
                    in_offset=IndirectOffsetOnAxis(ap=idx_c[q][:, j:j + 1], axis=0))
                cors.append(cq)
            xpre = gpool.tile([P, P], F32, tag="xpre")
            nc.vector.tensor_scalar(
                out=xpre[:, 0:Cf], in0=cors[0][:],
                scalar1=w00[:, j:j + 1], scalar2=None, op0=ALU.mult)
            for q in range(1, 4):
                nc.vector.scalar_tensor_tensor(
                    out=xpre[:, 0:Cf], in0=cors[q][:],
                    scalar=wcorner[q][:, j:j + 1], in1=xpre[:, 0:Cf],
                    op0=ALU.mult, op1=ALU.add)
            nc.vector.tensor_copy(out=xpre[:, Cf:Cf + 2], in_=v3[:, j, :])
            tp = psum.tile([P, P], F32, tag="tps", bufs=2)
            nc.tensor.transpose(out=tp[:], in_=xpre[:], identity=ident[:])
            nc.scalar.copy(out=x_in[:, j * P:(j + 1) * P], in_=tp[:])

        # ---- layer 0: 1x1 conv D->Ch + ReLU ----
        for co in range(CB):
            for b in range(BPC):
                for s in range(N // HALF):
                    ps = psum.tile([P, HALF], F32, tag="mm")
                    nc.tensor.matmul(
                        ps[:],
                        lhsT=w0_sb[:, co * P:(co + 1) * P],
                        rhs=x_in[:, b * N + s * HALF:b * N + (s + 1) * HALF],
                        start=True, stop=True)
                    nc.scalar.activation(
                        h[0][co][b][:, PAD + s * HALF:PAD + (s + 1) * HALF],
                        ps[:], AF.Relu, bias=b0_sb[:, co:co + 1])

        # ---- 6 dilated conv layers ----
        for li, dil in enumerate(DILS[:nlayers]):
            gin, gout = li % 2, (li + 1) % 2
            wt = wcur[li]
            for co in range(CB):
                for b in range(BPC):
                    for s in range(N // HALF):
                        ps = psum.tile([P, HALF], F32, tag="mm")
                        for ci in range(CB):
                            for k in range(3):
                                off = PAD + s * HALF + (k - 1) * dil
                                col = (k * CB + ci) * Ch + co * P
                                nc.tensor.matmul(
                                    ps[:],
                                    lhsT=wt[:, col:col + P],
                                    rhs=h[gin][ci][b][:, off:off + HALF],
                                    start=(ci == 0 and k == 0),
                                    stop=(ci == CB - 1 and k == 2))
                        nc.scalar.activation(
                            h[gout][co][b][:, PAD + s * HALF:PAD + (s + 1) * HALF],
                            ps[:], AF.Relu, bias=bs_sb[:, li * CB + co:li * CB + co + 1])

        # ---- final 1x1 conv Ch->2 (no bias) ----
        gfin = nlayers % 2
        out_sb = const.tile([2, T], F32)
        for b in range(BPC):
            for s in range(N // HALF):
                ps = psum.tile([2, HALF], F32, tag="fin", bufs=2)
                for ci in range(CB):
                    nc.tensor.matmul(
                        ps[:],
                        lhsT=woff_sb[:, ci * 2:(ci + 1) * 2],
                        rhs=h[gfin][ci][b][:, PAD + s * HALF:PAD + (s + 1) * HALF],
                        start=(ci == 0), stop=(ci == CB - 1))
                nc.vector.tensor_copy(
                    out=out_sb[:, b * N + s * HALF:b * N + (s + 1) * HALF], in_=ps[:])
        nc.sync.dma_start(out=out[:], in_=out_sb[:])


def shard_inputs(vertices, feature_map, w0, b0, ws, bs, w_off):
    """Build the per-core input maps (host-side repack, all cheap except fm)."""
    vertices = np.ascontiguousarray(vertices, np.float32)
    feature_map = np.ascontiguousarray(feature_map, np.float32)
    w0r = np.ascontiguousarray(w0.reshape(D, Ch), np.float32)
    b0r = np.ascontiguousarray(b0.reshape(CB, P).T, np.float32)
    # ws[l,k,ci*128+p,co] -> [l, p, (k ci co)]
    wsr = np.ascontiguousarray(
        ws.reshape(6, 3, CB, P, Ch).transpose(0, 3, 1, 2, 4).reshape(6, P, 3 * CB * Ch),
        np.float32)
    bsr = np.ascontiguousarray(
        bs.reshape(6, CB, P).transpose(2, 0, 1).reshape(P, 6 * CB), np.float32)
    woffr = np.ascontiguousarray(
        w_off.reshape(CB, P, 2).transpose(1, 0, 2).reshape(P, CB * 2), np.float32)

    in_maps = []
    for c in range(NCORES):
        vb = vertices[c * BPC:(c + 1) * BPC]          # [BPC, N, 2]
        vr = np.ascontiguousarray(
            vb.reshape(BPC, N // P, P, 2).transpose(2, 0, 1, 3).reshape(P, BPC * (N // P) * 2))
        fmb = feature_map[c * BPC:(c + 1) * BPC].reshape(BPC * H * W, Cf)
        in_maps.append({
            "verts": vr,
            "fm": np.ascontiguousarray(fmb),
            "w0": w0r, "b0": b0r, "ws": wsr, "bs": bsr, "woff": woffr,
        })
    return in_maps


def unshard_output(results):
    outs = []
    for r in results:
        o = np.asarray(r["out"])                       # [2, T] = [ch, b*N+n]
        outs.append(o.reshape(2, BPC, N).transpose(1, 2, 0))   # [BPC, N, 2]
    return np.concatenate(outs, axis=0).astype(np.float32)


_NC_CACHE = {}


def _get_program():
    if "nc" not in _NC_CACHE:
        _NC_CACHE["nc"] = build_program()
    return _NC_CACHE["nc"]


def run(inputs, trace=False):
    nc = _get_program()
    in_maps = shard_inputs(**inputs)
    res = run_bass_kernel_spmd(nc, in_maps, list(range(NCORES)), trace=trace)
    return unshard_output(res.results), res


def kernel(**inputs) -> np.ndarray:
    out, _ = run(inputs, trace=False)
    return out



# revision 4
# speedup vs baseline: 1.0172x; 1.0172x over previous
"""SnakeHead Trainium2 kernel.

Model (per batch): bilinear-sample a [256,256,126] feature map at 1024
vertices, concat the (y,x) coords -> [1024,128], 1x1 conv to 512 + ReLU,
six dilated (1,3,9,9,3,1) kernel-3 conv1d layers 512->512 + ReLU, final
1x1 conv 512->2.

Strategy: data-parallel over batch, 2 batches per NeuronCore (16/8).
On each core:
  - gather the 4 bilinear corners with 4 large multi-offset indirect
    DMAs (one per batch per y-level; corners (y,x0)/(y,x0+1) are
    contiguous rows in DRAM so one 252-element descriptor covers both)
  - bilinear-combine on the vector engine, vertices-on-partitions
  - PE-transpose to channel-major [ch, tok] layout, cast to bf16
  - all convs as bf16 matmuls (full PE rate, fp32 PSUM accumulate),
    channels on partitions, tokens on the free axis; dilated taps are
    shifted rhs slices into zero-padded halo activation buffers
  - bias+ReLU fused in the scalar-engine PSUM->SBUF eviction
"""

import numpy as np
from contextlib import ExitStack

import concourse.bass as bass
import concourse.bacc as bacc
import concourse.mybir as mybir
import concourse.tile as tile
from concourse.bass import IndirectOffsetOnAxis
from concourse.bass_utils import run_bass_kernel_spmd
from concourse.masks import make_identity

P = 128
B, N, H, W, Cf, Ch = 16, 1024, 256, 256, 126, 512
NCORES = 8
BPC = B // NCORES          # batches per core
T = BPC * N                # tokens per core
D = Cf + 2                 # input channels to layer 0
DILS = (1, 3, 9, 9, 3, 1)
PAD = 16                   # halo >= max dilation (9)
SEG = PAD + N + PAD        # per-batch activation columns
NT = T // P                # 128-token tiles per core
NTB = N // P               # 128-token tiles per batch (8)
CB = Ch // P               # 128-channel blocks (4)
HALF = 512                 # matmul moving-dim tile (tokens)
RPAD = BPC * H * W + 1     # fm rows + 1 pad row (pair-read overrun guard)

F32 = mybir.dt.float32
BF = mybir.dt.bfloat16
I32 = mybir.dt.int32
AF = mybir.ActivationFunctionType
ALU = mybir.AluOpType

NPBF = mybir.dt.np(BF)


def build_program(reps=1, nlayers=6, wbufs=2):
    nc = bacc.Bacc(trn_type="TRN2", target_bir_lowering=False)

    verts = nc.declare_dram_parameter("verts", [P, NT * 2], F32, False)
    fm = nc.declare_dram_parameter("fm", [RPAD, Cf], BF, False)
    w0 = nc.declare_dram_parameter("w0", [P, Ch], BF, False)
    b0 = nc.declare_dram_parameter("b0", [P, CB], F32, False)
    ws = nc.declare_dram_parameter("ws", [6, P, 3 * CB * Ch], BF, False)
    bs = nc.declare_dram_parameter("bs", [P, 6 * CB], F32, False)
    woff = nc.declare_dram_parameter("woff", [P, CB * 2], BF, False)
    out = nc.declare_dram_parameter("out", [2, T], F32, True)

    with tile.TileContext(nc) as tc, ExitStack() as ctx:
        const = ctx.enter_context(tc.tile_pool(name="const", bufs=1))
        work = ctx.enter_context(tc.tile_pool(name="work", bufs=1))
        gpool = ctx.enter_context(tc.tile_pool(name="gpool", bufs=4))
        wpool = ctx.enter_context(tc.tile_pool(name="wpool", bufs=wbufs))
        hpool = ctx.enter_context(tc.tile_pool(name="hpool", bufs=1))
        psum = ctx.enter_context(tc.tile_pool(name="psum", bufs=4, space="PSUM"))
        for _ in range(reps):
            _emit_body(nc, tc, const, work, gpool, wpool, hpool, psum,
                       verts, fm, w0, b0, ws, bs, woff, out, nlayers)

    # Clear all kernel semaphores + DMA queues at the tail so the loaded
    # NEFF can be re-executed: without this, a second nrt_execute starts
    # with end-of-run semaphore values and every wait is pre-satisfied.
    nc.reset()
    nc.finalize()
    return nc


def _emit_body(nc, tc, const, work, gpool, wpool, hpool, psum,
               verts, fm, w0, b0, ws, bs, woff, out, nlayers=6):
    if True:
        # ---- verts first (everything else waits on the vertex math) ----
        v_sb = const.tile([P, NT * 2], F32)                   # [p, (j) yx]
        nc.sync.dma_start(out=v_sb[:], in_=verts[:])

        # ---- vertex math: coords, floor, weights, flat indices ----
        v3 = v_sb[:].rearrange("p (j t) -> p j t", t=2)       # [128, 16, 2]
        cyx = work.tile([P, NT, 2], F32)
        # coords = (v+1) * (dim-1)/2
        nc.vector.tensor_scalar(
            out=cyx[:], in0=v3, scalar1=127.5, scalar2=127.5,
            op0=ALU.mult, op1=ALU.add)
        yi = work.tile([P, NT, 2], I32)
        nc.vector.tensor_copy(yi[:], cyx[:])                   # unknown rounding
        yf = work.tile([P, NT, 2], F32)
        nc.vector.tensor_copy(yf[:], yi[:])
        gt = work.tile([P, NT, 2], F32)
        nc.vector.tensor_tensor(out=gt[:], in0=yf[:], in1=cyx[:], op=ALU.is_gt)
        y0 = work.tile([P, NT, 2], F32)                        # floor(coords)
        nc.vector.tensor_tensor(out=y0[:], in0=yf[:], in1=gt[:], op=ALU.subtract)

        idxf = work.tile([P, NT], F32)                         # y0*W + x0 (+ batch)
        nc.vector.scalar_tensor_tensor(
            out=idxf[:], in0=y0[:, :, 0], scalar=float(W), in1=y0[:, :, 1],
            op0=ALU.mult, op1=ALU.add)
        for b in range(1, BPC):
            sl = idxf[:, b * NTB:(b + 1) * NTB]
            nc.vector.tensor_scalar_add(out=sl, in0=sl, scalar1=float(b * H * W))
        # pair-row indices: level 0 = row (y0,x0) (covers x0, x0+1),
        # level 1 = row (y0+1,x0)
        idx_l = []
        for li_, delta in enumerate((0.0, float(W))):
            idxi = work.tile([P, NT], I32, name=f"idxi{li_}")
            if delta == 0.0:
                nc.vector.tensor_copy(idxi[:], idxf[:])
            else:
                idxd = work.tile([P, NT], F32, name=f"idxd{li_}")
                nc.vector.tensor_scalar_add(out=idxd[:], in0=idxf[:], scalar1=delta)
                nc.vector.tensor_copy(idxi[:], idxd[:])
            idx_l.append(idxi)

        # ---- gather: one indirect DMA per (tile, y-level), [P,1] offsets ----
        # each descriptor reads 252 contiguous elements = corner pair
        # (y,x0),(y,x0+1); 128 descriptors per call.
        cpair = []
        for j in range(NT):
            row = []
            for lv in range(2):
                ct = gpool.tile([P, 2 * Cf], BF, name=f"cp{lv}", tag=f"cp{lv}",
                                bufs=8)
                nc.gpsimd.indirect_dma_start(
                    out=ct[:], out_offset=None, in_=fm[:],
                    in_offset=IndirectOffsetOnAxis(
                        ap=idx_l[lv][:, j:j + 1], axis=0))
                row.append(ct)
            cpair.append(row)

        # ---- bilinear weights ----
        wyx = work.tile([P, NT, 2], F32)                       # frac part
        nc.vector.tensor_tensor(out=wyx[:], in0=cyx[:], in1=y0[:], op=ALU.subtract)
        uyx = work.tile([P, NT, 2], F32)                       # 1 - frac
        nc.vector.tensor_scalar(
            out=uyx[:], in0=wyx[:], scalar1=-1.0, scalar2=1.0,
            op0=ALU.mult, op1=ALU.add)
        wy, wx = wyx[:, :, 0], wyx[:, :, 1]
        uy, ux = uyx[:, :, 0], uyx[:, :, 1]
        w00 = work.tile([P, NT], F32)
        w01 = work.tile([P, NT], F32)
        w10 = work.tile([P, NT], F32)
        w11 = work.tile([P, NT], F32)
        nc.vector.tensor_tensor(out=w00[:], in0=uy, in1=ux, op=ALU.mult)
        nc.vector.tensor_tensor(out=w01[:], in0=uy, in1=wx, op=ALU.mult)
        nc.vector.tensor_tensor(out=w10[:], in0=wy, in1=ux, op=ALU.mult)
        nc.vector.tensor_tensor(out=w11[:], in0=wy, in1=wx, op=ALU.mult)

        # ---- small loads / constants ----
        ident = const.tile([P, P], F32)
        make_identity(nc, ident[:])
        w0_sb = const.tile([P, Ch], BF)
        nc.sync.dma_start(out=w0_sb[:], in_=w0[:])
        b0_sb = const.tile([P, CB], F32)
        nc.sync.dma_start(out=b0_sb[:], in_=b0[:])
        bs_sb = const.tile([P, 6 * CB], F32)
        nc.sync.dma_start(out=bs_sb[:], in_=bs[:])
        woff_sb = const.tile([P, CB * 2], BF)
        nc.sync.dma_start(out=woff_sb[:], in_=woff[:])

        # ---- activation halo buffers (ping/pong) ----
        h = [[[hpool.tile([P, SEG], BF, name=f"h{g}_{ci}_{b}", tag=f"h{g}_{ci}_{b}")
               for b in range(BPC)] for ci in range(CB)] for g in range(2)]
        zeros_bf = const.tile([P, PAD], BF)
        nc.vector.memset(zeros_bf[:], 0.0)
        for g in range(2):
            for ci in range(CB):
                for b in range(BPC):
                    nc.vector.tensor_copy(h[g][ci][b][:, 0:PAD], zeros_bf[:])
                    nc.vector.tensor_copy(h[g][ci][b][:, PAD + N:SEG], zeros_bf[:])

        # ---- layer weight prefetch (double buffered) ----
        wcur = []
        for li in range(nlayers):
            wt = wpool.tile([P, 3 * CB * Ch], BF, tag="wlayer")
            nc.sync.dma_start(out=wt[:], in_=ws[li])
            wcur.append(wt)

        # ---- per batch: bilinear + transpose into x_in, then layer 0 ----
        x_in = const.tile([P, T], BF)
        for b in range(BPC):
            for jj in range(NTB):
                j = b * NTB + jj
                c0, c1 = cpair[j][0], cpair[j][1]
                xpre = gpool.tile([P, P], F32, tag="xpre")
                nc.vector.tensor_scalar(
                    out=xpre[:, 0:Cf], in0=c0[:, 0:Cf],
                    scalar1=w00[:, j:j + 1], scalar2=None, op0=ALU.mult)
                for cq, wq in ((c0[:, Cf:2 * Cf], w01),
                               (c1[:, 0:Cf], w10),
                               (c1[:, Cf:2 * Cf], w11)):
                    nc.vector.scalar_tensor_tensor(
                        out=xpre[:, 0:Cf], in0=cq,
                        scalar=wq[:, j:j + 1], in1=xpre[:, 0:Cf],
                        op0=ALU.mult, op1=ALU.add)
                nc.vector.tensor_copy(out=xpre[:, Cf:Cf + 2], in_=v3[:, j, :])
                tp = psum.tile([P, P], F32, tag="tps", bufs=2)
                nc.tensor.transpose(out=tp[:], in_=xpre[:], identity=ident[:])
                nc.scalar.copy(out=x_in[:, j * P:(j + 1) * P], in_=tp[:])

            # layer 0: 1x1 conv D->Ch + ReLU for this batch
            for co in range(CB):
                for s in range(N // HALF):
                    ps = psum.tile([P, HALF], F32, tag="mm")
                    nc.tensor.matmul(
                        ps[:],
                        lhsT=w0_sb[:, co * P:(co + 1) * P],
                        rhs=x_in[:, b * N + s * HALF:b * N + (s + 1) * HALF],
                        start=True, stop=True)
                    nc.scalar.activation(
                        h[0][co][b][:, PAD + s * HALF:PAD + (s + 1) * HALF],
                        ps[:], AF.Relu, bias=b0_sb[:, co:co + 1])

        # ---- 6 dilated conv layers ----
        for li, dil in enumerate(DILS[:nlayers]):
            gin, gout = li % 2, (li + 1) % 2
            wt = wcur[li]
            for co in range(CB):
                for b in range(BPC):
                    for s in range(N // HALF):
                        ps = psum.tile([P, HALF], F32, tag="mm")
                        for ci in range(CB):
                            for k in range(3):
                                off = PAD + s * HALF + (k - 1) * dil
                                col = (k * CB + ci) * Ch + co * P
                                nc.tensor.matmul(
                                    ps[:],
                                    lhsT=wt[:, col:col + P],
                                    rhs=h[gin][ci][b][:, off:off + HALF],
                                    start=(ci == 0 and k == 0),
                                    stop=(ci == CB - 1 and k == 2))
                        nc.scalar.activation(
                            h[gout][co][b][:, PAD + s * HALF:PAD + (s + 1) * HALF],
                            ps[:], AF.Relu, bias=bs_sb[:, li * CB + co:li * CB + co + 1])

        # ---- final 1x1 conv Ch->2 (no bias), output DMA per batch ----
        gfin = nlayers % 2
        out_sb = const.tile([2, T], F32)
        for b in range(BPC):
            for s in range(N // HALF):
                ps = psum.tile([2, HALF], F32, tag="fin", bufs=2)
                for ci in range(CB):
                    nc.tensor.matmul(
                        ps[:],
                        lhsT=woff_sb[:, ci * 2:(ci + 1) * 2],
                        rhs=h[gfin][ci][b][:, PAD + s * HALF:PAD + (s + 1) * HALF],
                        start=(ci == 0), stop=(ci == CB - 1))
                nc.vector.tensor_copy(
                    out=out_sb[:, b * N + s * HALF:b * N + (s + 1) * HALF], in_=ps[:])
            nc.sync.dma_start(out=out[:, b * N:(b + 1) * N],
                              in_=out_sb[:, b * N:(b + 1) * N])


def shard_inputs(vertices, feature_map, w0, b0, ws, bs, w_off):
    """Build the per-core input maps (host-side repack, all cheap except fm)."""
    vertices = np.ascontiguousarray(vertices, np.float32)
    fm_bf = np.asarray(feature_map, np.float32).astype(NPBF)   # [B,H,W,Cf]
    w0r = np.ascontiguousarray(np.asarray(w0, np.float32).reshape(D, Ch)).astype(NPBF)
    b0r = np.ascontiguousarray(np.asarray(b0, np.float32).reshape(CB, P).T, np.float32)
    # ws[l,k,ci*128+p,co] -> [l, p, (k ci co)]
    wsr = np.ascontiguousarray(
        np.asarray(ws, np.float32)
        .reshape(6, 3, CB, P, Ch).transpose(0, 3, 1, 2, 4).reshape(6, P, 3 * CB * Ch)
    ).astype(NPBF)
    bsr = np.ascontiguousarray(
        np.asarray(bs, np.float32).reshape(6, CB, P).transpose(2, 0, 1).reshape(P, 6 * CB),
        np.float32)
    woffr = np.ascontiguousarray(
        np.asarray(w_off, np.float32).reshape(CB, P, 2).transpose(1, 0, 2).reshape(P, CB * 2)
    ).astype(NPBF)

    in_maps = []
    for c in range(NCORES):
        vb = vertices[c * BPC:(c + 1) * BPC]          # [BPC, N, 2]
        vr = np.ascontiguousarray(
            vb.reshape(BPC, NTB, P, 2).transpose(2, 0, 1, 3).reshape(P, NT * 2))
        fmb = np.empty((RPAD, Cf), NPBF)
        fmb[:BPC * H * W] = fm_bf[c * BPC:(c + 1) * BPC].reshape(BPC * H * W, Cf)
        fmb[BPC * H * W:] = 0
        in_maps.append({
            "verts": vr,
            "fm": fmb,
            "w0": w0r, "b0": b0r, "ws": wsr, "bs": bsr, "woff": woffr,
        })
    return in_maps


def unshard_output(results):
    outs = []
    for r in results:
        o = np.asarray(r["out"])                       # [2, T] = [ch, b*N+n]
        outs.append(o.reshape(2, BPC, N).transpose(1, 2, 0))   # [BPC, N, 2]
    return np.concatenate(outs, axis=0).astype(np.float32)


_NC_CACHE = {}


def _get_program():
    if "nc" not in _NC_CACHE:
        _NC_CACHE["nc"] = build_program()
    return _NC_CACHE["nc"]


def run(inputs, trace=False):
    nc = _get_program()
    in_maps = shard_inputs(**inputs)
    res = run_bass_kernel_spmd(nc, in_maps, list(range(NCORES)), trace=trace)
    return unshard_output(res.results), res


def kernel(**inputs) -> np.ndarray:
    out, _ = run(inputs, trace=False)
    return out


# revision 5
# speedup vs baseline: 1.1840x; 1.1639x over previous
"""SnakeHead Trainium2 kernel.

Model (per batch): bilinear-sample a [256,256,126] feature map at 1024
vertices, concat the (y,x) coords -> [1024,128], 1x1 conv to 512 + ReLU,
six dilated (1,3,9,9,3,1) kernel-3 conv1d layers 512->512 + ReLU, final
1x1 conv 512->2.

Strategy: data-parallel over batch, 2 batches per NeuronCore (16/8).
On each core:
  - gather the 4 bilinear corners with 32 indirect DMAs ([P,1] offsets,
    one per 128-token tile per y-level); corners (y,x0)/(y,x0+1) are
    contiguous rows in DRAM so one 252-element (1008 B) descriptor
    covers both
  - bilinear-combine on the vector engine, vertices-on-partitions
  - PE-transpose to channel-major [ch, tok] layout
  - all convs as float32r matmuls (full PE rate), channels on
    partitions, tokens on the free axis; dilated taps are just shifted
    rhs slices into zero-padded halo activation buffers
  - bias+ReLU fused in the scalar-engine PSUM->SBUF eviction
  - emission order keeps the PE queue stall-free: batch 0's transposes/
    layer0/layer1 run while batch 1's gather is still in flight
"""

import numpy as np
from contextlib import ExitStack

import concourse.bass as bass
import concourse.bacc as bacc
import concourse.mybir as mybir
import concourse.tile as tile
from concourse.bass import IndirectOffsetOnAxis
from concourse.bass_utils import run_bass_kernel_spmd
from concourse.masks import make_identity

P = 128
B, N, H, W, Cf, Ch = 16, 1024, 256, 256, 126, 512
NCORES = 8
BPC = B // NCORES          # batches per core
T = BPC * N                # tokens per core
D = Cf + 2                 # input channels to layer 0
DILS = (1, 3, 9, 9, 3, 1)
PAD = 16                   # halo >= max dilation (9)
SEG = PAD + N + PAD        # per-batch activation columns
NT = T // P                # 128-token tiles per core
NTB = N // P               # 128-token tiles per batch (8)
CB = Ch // P               # 128-channel blocks (4)
HALF = 512                 # matmul moving-dim tile (tokens)
RPAD = BPC * H * W + 1     # fm rows + 1 pad row (pair-read overrun guard)

F32 = mybir.dt.float32
FR = mybir.dt.float32r
I32 = mybir.dt.int32
AF = mybir.ActivationFunctionType
ALU = mybir.AluOpType


def build_program(reps=1, nlayers=6, wbufs=2):
    nc = bacc.Bacc(trn_type="TRN2", target_bir_lowering=False)

    verts = nc.declare_dram_parameter("verts", [P, NT * 2], F32, False)
    fm = nc.declare_dram_parameter("fm", [RPAD, Cf], F32, False)
    w0 = nc.declare_dram_parameter("w0", [P, Ch], FR, False)
    b0 = nc.declare_dram_parameter("b0", [P, CB], F32, False)
    ws = nc.declare_dram_parameter("ws", [6, P, 3 * CB * Ch], FR, False)
    bs = nc.declare_dram_parameter("bs", [P, 6 * CB], F32, False)
    woff = nc.declare_dram_parameter("woff", [P, CB * 2], FR, False)
    out = nc.declare_dram_parameter("out", [2, T], F32, True)

    with tile.TileContext(nc) as tc, ExitStack() as ctx:
        const = ctx.enter_context(tc.tile_pool(name="const", bufs=1))
        work = ctx.enter_context(tc.tile_pool(name="work", bufs=1))
        gpool = ctx.enter_context(tc.tile_pool(name="gpool", bufs=4))
        wpool = ctx.enter_context(tc.tile_pool(name="wpool", bufs=wbufs))
        hpool = ctx.enter_context(tc.tile_pool(name="hpool", bufs=1))
        psum = ctx.enter_context(tc.tile_pool(name="psum", bufs=4, space="PSUM"))
        for _ in range(reps):
            _emit_body(nc, tc, const, work, gpool, wpool, hpool, psum,
                       verts, fm, w0, b0, ws, bs, woff, out, nlayers)

    # Clear all kernel semaphores + DMA queues at the tail so the loaded
    # NEFF can be re-executed: without this, a second nrt_execute starts
    # with end-of-run semaphore values and every wait is pre-satisfied.
    nc.reset()
    nc.finalize()
    return nc


def _emit_body(nc, tc, const, work, gpool, wpool, hpool, psum,
               verts, fm, w0, b0, ws, bs, woff, out, nlayers=6):
    if True:
        # ---- verts first (everything else waits on the vertex math) ----
        v_sb = const.tile([P, NT * 2], F32)                   # [p, (j) yx]
        nc.sync.dma_start(out=v_sb[:], in_=verts[:])

        # ---- vertex math: coords, floor, weights, flat indices ----
        v3 = v_sb[:].rearrange("p (j t) -> p j t", t=2)       # [128, 16, 2]
        cyx = work.tile([P, NT, 2], F32)
        # coords = (v+1) * (dim-1)/2
        nc.vector.tensor_scalar(
            out=cyx[:], in0=v3, scalar1=127.5, scalar2=127.5,
            op0=ALU.mult, op1=ALU.add)
        yi = work.tile([P, NT, 2], I32)
        nc.vector.tensor_copy(yi[:], cyx[:])                   # unknown rounding
        yf = work.tile([P, NT, 2], F32)
        nc.vector.tensor_copy(yf[:], yi[:])
        gt = work.tile([P, NT, 2], F32)
        nc.vector.tensor_tensor(out=gt[:], in0=yf[:], in1=cyx[:], op=ALU.is_gt)
        y0 = work.tile([P, NT, 2], F32)                        # floor(coords)
        nc.vector.tensor_tensor(out=y0[:], in0=yf[:], in1=gt[:], op=ALU.subtract)

        idxf = work.tile([P, NT], F32)                         # y0*W + x0 (+ batch)
        nc.vector.scalar_tensor_tensor(
            out=idxf[:], in0=y0[:, :, 0], scalar=float(W), in1=y0[:, :, 1],
            op0=ALU.mult, op1=ALU.add)
        for b in range(1, BPC):
            sl = idxf[:, b * NTB:(b + 1) * NTB]
            nc.vector.tensor_scalar_add(out=sl, in0=sl, scalar1=float(b * H * W))
        # pair-row indices: level 0 = row (y0,x0) (covers x0, x0+1),
        # level 1 = row (y0+1,x0)
        idx_l = []
        for li_, delta in enumerate((0.0, float(W))):
            idxi = work.tile([P, NT], I32, name=f"idxi{li_}")
            if delta == 0.0:
                nc.vector.tensor_copy(idxi[:], idxf[:])
            else:
                idxd = work.tile([P, NT], F32, name=f"idxd{li_}")
                nc.vector.tensor_scalar_add(out=idxd[:], in0=idxf[:], scalar1=delta)
                nc.vector.tensor_copy(idxi[:], idxd[:])
            idx_l.append(idxi)

        # ---- gather: one indirect DMA per (tile, y-level), [P,1] offsets ----
        # each descriptor reads 252 contiguous f32 = corner pair
        # (y,x0),(y,x0+1); 128 descriptors (1008 B each) per call.
        cpair = []
        for j in range(NT):
            row = []
            for lv in range(2):
                ct = gpool.tile([P, 2 * Cf], F32, name=f"cp{lv}", tag=f"cp{lv}",
                                bufs=8)
                nc.gpsimd.indirect_dma_start(
                    out=ct[:], out_offset=None, in_=fm[:],
                    in_offset=IndirectOffsetOnAxis(
                        ap=idx_l[lv][:, j:j + 1], axis=0))
                row.append(ct)
            cpair.append(row)

        # ---- bilinear weights ----
        wyx = work.tile([P, NT, 2], F32)                       # frac part
        nc.vector.tensor_tensor(out=wyx[:], in0=cyx[:], in1=y0[:], op=ALU.subtract)
        uyx = work.tile([P, NT, 2], F32)                       # 1 - frac
        nc.vector.tensor_scalar(
            out=uyx[:], in0=wyx[:], scalar1=-1.0, scalar2=1.0,
            op0=ALU.mult, op1=ALU.add)
        wy, wx = wyx[:, :, 0], wyx[:, :, 1]
        uy, ux = uyx[:, :, 0], uyx[:, :, 1]
        w00 = work.tile([P, NT], F32)
        w01 = work.tile([P, NT], F32)
        w10 = work.tile([P, NT], F32)
        w11 = work.tile([P, NT], F32)
        nc.vector.tensor_tensor(out=w00[:], in0=uy, in1=ux, op=ALU.mult)
        nc.vector.tensor_tensor(out=w01[:], in0=uy, in1=wx, op=ALU.mult)
        nc.vector.tensor_tensor(out=w10[:], in0=wy, in1=ux, op=ALU.mult)
        nc.vector.tensor_tensor(out=w11[:], in0=wy, in1=wx, op=ALU.mult)

        # ---- small loads / constants ----
        ident = const.tile([P, P], F32)
        make_identity(nc, ident[:])
        w0_sb = const.tile([P, Ch], FR)
        nc.sync.dma_start(out=w0_sb[:], in_=w0[:])
        b0_sb = const.tile([P, CB], F32)
        nc.sync.dma_start(out=b0_sb[:], in_=b0[:])
        bs_sb = const.tile([P, 6 * CB], F32)
        nc.sync.dma_start(out=bs_sb[:], in_=bs[:])
        woff_sb = const.tile([P, CB * 2], FR)
        nc.sync.dma_start(out=woff_sb[:], in_=woff[:])

        # ---- activation halo buffers (ping/pong) ----
        h = [[[hpool.tile([P, SEG], FR, name=f"h{g}_{ci}_{b}", tag=f"h{g}_{ci}_{b}")
               for b in range(BPC)] for ci in range(CB)] for g in range(2)]
        zeros_f32 = const.tile([P, PAD], F32)
        nc.vector.memset(zeros_f32[:], 0.0)
        for g in range(2):
            for ci in range(CB):
                for b in range(BPC):
                    nc.vector.tensor_copy(h[g][ci][b][:, 0:PAD], zeros_f32[:])
                    nc.vector.tensor_copy(h[g][ci][b][:, PAD + N:SEG], zeros_f32[:])

        # ---- layer weight prefetch (double buffered) ----
        wcur = []
        for li in range(nlayers):
            wt = wpool.tile([P, 3 * CB * Ch], FR, tag="wlayer")
            nc.sync.dma_start(out=wt[:], in_=ws[li])
            wcur.append(wt)

        x_in = const.tile([P, T], FR)

        def emit_xin_l0(b):
            """bilinear + transpose into x_in for batch b, then layer 0."""
            for jj in range(NTB):
                j = b * NTB + jj
                c0, c1 = cpair[j][0], cpair[j][1]
                xpre = gpool.tile([P, P], F32, tag="xpre")
                nc.vector.tensor_scalar(
                    out=xpre[:, 0:Cf], in0=c0[:, 0:Cf],
                    scalar1=w00[:, j:j + 1], scalar2=None, op0=ALU.mult)
                for cq, wq in ((c0[:, Cf:2 * Cf], w01),
                               (c1[:, 0:Cf], w10),
                               (c1[:, Cf:2 * Cf], w11)):
                    nc.vector.scalar_tensor_tensor(
                        out=xpre[:, 0:Cf], in0=cq,
                        scalar=wq[:, j:j + 1], in1=xpre[:, 0:Cf],
                        op0=ALU.mult, op1=ALU.add)
                nc.vector.tensor_copy(out=xpre[:, Cf:Cf + 2], in_=v3[:, j, :])
                tp = psum.tile([P, P], F32, tag="tps", bufs=2)
                nc.tensor.transpose(out=tp[:], in_=xpre[:], identity=ident[:])
                nc.scalar.copy(out=x_in[:, j * P:(j + 1) * P], in_=tp[:])

            for co in range(CB):
                for s in range(N // HALF):
                    ps = psum.tile([P, HALF], F32, tag="mm")
                    nc.tensor.matmul(
                        ps[:],
                        lhsT=w0_sb[:, co * P:(co + 1) * P],
                        rhs=x_in[:, b * N + s * HALF:b * N + (s + 1) * HALF],
                        start=True, stop=True)
                    nc.scalar.activation(
                        h[0][co][b][:, PAD + s * HALF:PAD + (s + 1) * HALF],
                        ps[:], AF.Relu, bias=b0_sb[:, co:co + 1])

        def emit_layer(li, dil, b):
            gin, gout = li % 2, (li + 1) % 2
            wt = wcur[li]
            for co in range(CB):
                for s in range(N // HALF):
                    ps = psum.tile([P, HALF], F32, tag="mm")
                    for ci in range(CB):
                        for k in range(3):
                            off = PAD + s * HALF + (k - 1) * dil
                            col = (k * CB + ci) * Ch + co * P
                            nc.tensor.matmul(
                                ps[:],
                                lhsT=wt[:, col:col + P],
                                rhs=h[gin][ci][b][:, off:off + HALF],
                                start=(ci == 0 and k == 0),
                                stop=(ci == CB - 1 and k == 2))
                    nc.scalar.activation(
                        h[gout][co][b][:, PAD + s * HALF:PAD + (s + 1) * HALF],
                        ps[:], AF.Relu, bias=bs_sb[:, li * CB + co:li * CB + co + 1])

        # ---- emission order: keep the PE queue stall-free ----
        # b0's x_in/L0/L1 run while b1's gather is still in flight; from
        # layer 2 on, batches alternate within each layer (single weight
        # load per layer, wpool-staged).
        emit_xin_l0(0)
        if nlayers > 0:
            emit_layer(0, DILS[0], 0)
        emit_xin_l0(1)
        if nlayers > 0:
            emit_layer(0, DILS[0], 1)
        for li, dil in enumerate(DILS[:nlayers]):
            if li == 0:
                continue
            for b in range(BPC):
                emit_layer(li, dil, b)

        # ---- final 1x1 conv Ch->2 (no bias), output DMA per batch ----
        gfin = nlayers % 2
        out_sb = const.tile([2, T], F32)
        for b in range(BPC):
            for s in range(N // HALF):
                ps = psum.tile([2, HALF], F32, tag="fin", bufs=2)
                for ci in range(CB):
                    nc.tensor.matmul(
                        ps[:],
                        lhsT=woff_sb[:, ci * 2:(ci + 1) * 2],
                        rhs=h[gfin][ci][b][:, PAD + s * HALF:PAD + (s + 1) * HALF],
                        start=(ci == 0), stop=(ci == CB - 1))
                nc.vector.tensor_copy(
                    out=out_sb[:, b * N + s * HALF:b * N + (s + 1) * HALF], in_=ps[:])
            nc.sync.dma_start(out=out[:, b * N:(b + 1) * N],
                              in_=out_sb[:, b * N:(b + 1) * N])


def shard_inputs(vertices, feature_map, w0, b0, ws, bs, w_off):
    """Build the per-core input maps (host-side repack, all cheap except fm)."""
    vertices = np.ascontiguousarray(vertices, np.float32)
    feature_map = np.asarray(feature_map, np.float32)
    w0r = np.ascontiguousarray(np.asarray(w0, np.float32).reshape(D, Ch))
    b0r = np.ascontiguousarray(np.asarray(b0, np.float32).reshape(CB, P).T)
    # ws[l,k,ci*128+p,co] -> [l, p, (k ci co)]
    wsr = np.ascontiguousarray(
        np.asarray(ws, np.float32)
        .reshape(6, 3, CB, P, Ch).transpose(0, 3, 1, 2, 4).reshape(6, P, 3 * CB * Ch))
    bsr = np.ascontiguousarray(
        np.asarray(bs, np.float32).reshape(6, CB, P).transpose(2, 0, 1).reshape(P, 6 * CB))
    woffr = np.ascontiguousarray(
        np.asarray(w_off, np.float32).reshape(CB, P, 2).transpose(1, 0, 2).reshape(P, CB * 2))

    in_maps = []
    for c in range(NCORES):
        vb = vertices[c * BPC:(c + 1) * BPC]          # [BPC, N, 2]
        vr = np.ascontiguousarray(
            vb.reshape(BPC, NTB, P, 2).transpose(2, 0, 1, 3).reshape(P, NT * 2))
        fmb = np.empty((RPAD, Cf), np.float32)
        fmb[:BPC * H * W] = feature_map[c * BPC:(c + 1) * BPC].reshape(BPC * H * W, Cf)
        fmb[BPC * H * W:] = 0
        in_maps.append({
            "verts": vr,
            "fm": fmb,
            "w0": w0r, "b0": b0r, "ws": wsr, "bs": bsr, "woff": woffr,
        })
    return in_maps


def unshard_output(results):
    outs = []
    for r in results:
        o = np.asarray(r["out"])                       # [2, T] = [ch, b*N+n]
        outs.append(o.reshape(2, BPC, N).transpose(1, 2, 0))   # [BPC, N, 2]
    return np.concatenate(outs, axis=0).astype(np.float32)


_NC_CACHE = {}


def _get_program():
    if "nc" not in _NC_CACHE:
        _NC_CACHE["nc"] = build_program()
    return _NC_CACHE["nc"]


def run(inputs, trace=False):
    nc = _get_program()
    in_maps = shard_inputs(**inputs)
    res = run_bass_kernel_spmd(nc, in_maps, list(range(NCORES)), trace=trace)
    return unshard_output(res.results), res


def kernel(**inputs) -> np.ndarray:
    out, _ = run(inputs, trace=False)
    return out
